# revision 1
# baseline (speedup 1.0000x reference)
"""3-layer GCN (PyG gcn_norm semantics) on 8 Trainium2 NeuronCores.

Sharding: nodes are range-partitioned across the 8 cores (graph parallel).
Each core owns rows [c*S, (c+1)*S) of every activation and of the
segment-sum output.  The small weight matrices are replicated.  Per layer:

  1. every core computes the dense transform h = a @ W for its own node
     range; the shards are AllGathered (in `pieces` slices, overlapped
     with compute) into a full "gather table" in each core's HBM,
  2. each core processes the edges whose *destination* lands in its range:
     an indirect DMA gathers h[src] rows (batches of GATHER_K*128 edges),
     a host-precomputed norm-weighted one-hot block
     onehot[e, w] = (dstlocal[e] == w) * norm[e] is streamed from DRAM for
     each 128-edge block (pure graph structure, fp16, reused all layers),
     and a PE matmul psum[feat, w] += gathered[e, feat].T @ onehot[e, w]
     accumulates messages into the PSUM tile of the current cw-row
     destination chunk (edges are pre-sorted by destination on the host,
     so each chunk's edges are contiguous blocks),
  3. the chunk epilogue applies bias/ReLU and the *next* layer's weight
     matmul while the data is on-chip, writing the next gather table shard.

The gather tables travel in `msg_dtype` (fp16 by default: gathers, the
one-hot stream and the message matmul run at 1-pass PE rate and half
DMA volume); all accumulation (PSUM), the epilogue, weights, and the
output stay fp32.  Chunk epilogues are batched (8 at a time) so PE is
not latency-chained through ACT between chunks.  The final layer computes logits + log_softmax per
chunk (exp/ln batched across chunks to avoid ACT LUT swaps) and each
core returns its row shard; the host concatenates.

Host-side work is limited to sharding/index preprocessing: edge
partitioning by dst range, sort by dst, degree counting, normalization
coefficients, and packing block-padded index/coefficient arrays.
All dense float math (matmuls, gathers, segment sums, activations,
log_softmax) runs on the NeuronCores.
"""

import os

import numpy as np

import concourse.bass as bass
import concourse.bacc as bacc
import concourse.mybir as mybir
import concourse.tile as tile
from concourse.bass import ts
from concourse.bass_utils import run_bass_kernel_spmd

F32 = mybir.dt.float32
F16 = mybir.dt.float16
I32 = mybir.dt.int32
P = 128  # partition dim == feature dim

LAST_RESULT = None


class Cfg:
    def __init__(self, n_cores, n_nodes, n_class, gather_k, pieces,
                 msg_dtype=mybir.dt.float8e4, cw=64):
        assert n_nodes % n_cores == 0
        self.n_cores = n_cores
        self.n_nodes = n_nodes
        self.n_class = n_class
        self.gather_k = gather_k
        self.cw = cw                         # scatter window (chunk) width
        self.S = n_nodes // n_cores          # rows per core
        self.CH = -(-self.S // cw)           # cw-row chunks per core
        self.S_pad = self.CH * cw
        assert self.S_pad % P == 0
        self.pad = self.S_pad - self.S
        self.T_pad = n_cores * self.S_pad    # padded gather-table rows
        assert self.CH % pieces == 0
        self.pieces = pieces
        self.piece_rows = self.S_pad // pieces
        self.msg_dtype = msg_dtype

    @property
    def np_msg(self):
        return np.dtype(mybir.dt.np(self.msg_dtype))


FULL = Cfg(n_cores=8, n_nodes=100000, n_class=10, gather_k=32, pieces=7)


def _table_row(cfg, i):
    """Map global node id -> row in the piece-major gather table."""
    c = i // cfg.S
    r = i - c * cfg.S
    p = r // cfg.piece_rows
    return (
        p * (cfg.n_cores * cfg.piece_rows)
        + c * cfg.piece_rows
        + (r - p * cfg.piece_rows)
    )


def _preprocess(cfg, edge_index):
    """Shard + sort edges by destination, build block-padded device arrays.

    Returns (NB, blocks, per_core) where blocks[b] = (chunk_id, first, last)
    and per_core is a list of dicts with idx/meta arrays per core.
    """
    S, CH, K = cfg.S, cfg.CH, cfg.gather_k
    n = cfg.n_nodes
    src = np.concatenate([edge_index[0], np.arange(n, dtype=np.int64)])
    dst = np.concatenate([edge_index[1], np.arange(n, dtype=np.int64)])
    deg = np.bincount(dst, minlength=n).astype(np.float64)
    dis = 1.0 / np.sqrt(deg)
    val = (dis[src] * dis[dst]).astype(np.float32)

    core = dst // S
    srcp = _table_row(cfg, src).astype(np.int32)
    dloc = dst - core * S
    chunk = dloc // cfg.cw
    w = (dloc % cfg.cw).astype(np.float32)

    G = cfg.n_cores * CH
    cc = (core * CH + chunk).astype(np.int64)
    counts = np.bincount(cc, minlength=G).reshape(cfg.n_cores, CH)
    Bc = np.maximum(1, -(-counts.max(axis=0) // P)).astype(np.int64)  # [CH]
    NB = int(Bc.sum())
    Bc[-1] += (-NB) % K
    NB = int(Bc.sum())

    chunk_off = np.zeros(CH, np.int64)
    chunk_off[1:] = np.cumsum(Bc * P)[:-1]
    L = NB * P

    # slot assignment: edges of (core, chunk) go to chunk_off[chunk] + rank
    order = np.argsort(cc, kind="stable")
    scc = cc[order]
    gstart = np.searchsorted(scc, np.arange(G))
    ranks = np.arange(len(order)) - gstart[scc]
    slots = chunk_off[scc % CH] + ranks
    cores_sorted = scc // CH

    md = cfg.np_msg
    idx_a = np.zeros((cfg.n_cores, L), np.int32)
    val_a = np.zeros((cfg.n_cores, L), np.float32)
    w_a = np.full((cfg.n_cores, L), -1, np.int64)
    idx_a[cores_sorted, slots] = srcp[order]
    val_a[cores_sorted, slots] = val[order]
    w_a[cores_sorted, slots] = w[order].astype(np.int64)

    blocks = []
    for c in range(CH):
        nb = int(Bc[c])
        for i in range(nb):
            blocks.append((c, i == 0, i == nb - 1))
    assert len(blocks) == NB

    per_core = []
    ar = np.arange(L)
    for c in range(cfg.n_cores):
        # one-hot scatter blocks: oh[e, w] = (w == wloc[e]) * val[e],
        # laid out for the device as [128 partitions(e), NB*128(b, w)]
        oh = np.zeros((L, cfg.cw), md)
        m = w_a[c] >= 0
        oh[ar[m], w_a[c][m]] = val_a[c][m].astype(md)
        oh = np.ascontiguousarray(
            oh.reshape(NB, P, cfg.cw).transpose(1, 0, 2).reshape(P, NB * cfg.cw)
        )
        # device tile layout: idx[p, b] = srcp of edge slot b*128+p
        per_core.append({
            "eidx": np.ascontiguousarray(idx_a[c].reshape(NB, P).T),
            "eoh": oh,
        })
    return NB, blocks, per_core


def _build_program(cfg, NB, blocks):
    nc = bacc.Bacc(
        "TRN2", target_bir_lowering=False, debug=False, num_devices=cfg.n_cores
    )
    CH, K, NC = cfg.CH, cfg.gather_k, cfg.n_class
    CW = cfg.cw
    MD = cfg.msg_dtype
    NG = NB // K
    rg = [list(range(cfg.n_cores))]
    cpp = CH // cfg.pieces          # chunks per AllGather piece
    prow = cfg.piece_rows

    # kernel I/O
    xT_in = nc.dram_tensor("xT", [P, cfg.S_pad], F16, kind="ExternalInput")
    eidx_in = nc.dram_tensor("eidx", [P, NB], I32, kind="ExternalInput")
    eoh_in = nc.dram_tensor("eoh", [P, NB * CW], MD, kind="ExternalInput")
    W_in = [
        nc.dram_tensor(f"W{i + 1}", [P, P], F16, kind="ExternalInput")
        for i in range(3)
    ]
    Wl_in = nc.dram_tensor("Wl", [P, NC], F16, kind="ExternalInput")
    b_in = [
        nc.dram_tensor(f"b{i + 1}", [P, 1], F32, kind="ExternalInput")
        for i in range(3)
    ]
    blT_in = nc.dram_tensor("blT", [P, NC], F32, kind="ExternalInput")
    out_t = nc.dram_tensor("logits", [CW, CH * NC], F32, kind="ExternalOutput")

    with tile.TileContext(nc) as tc:
        with (
            tc.tile_pool(name="const", bufs=1) as constp,
            tc.tile_pool(name="persist", bufs=1) as persist,
            tc.tile_pool(name="gather", bufs=6) as gatherp,
            tc.tile_pool(name="ohp", bufs=6) as ohp,
            tc.tile_pool(name="epi", bufs=3) as epip,
            tc.tile_pool(name="lsp", bufs=2) as lsp,
            tc.tile_pool(name="mpsum", bufs=3, space="PSUM") as mpsump,
            tc.tile_pool(name="opsum", bufs=4, space="PSUM") as opsump,
            tc.tile_pool(name="dram", bufs=1, space="DRAM") as dramp,
        ):
            W_t = []
            for i in range(3):
                wt = constp.tile([P, P], F16, name=f"w{i}")
                nc.sync.dma_start(wt[:], W_in[i][:])
                W_t.append(wt)
            Wl_t = constp.tile([P, NC], F16)
            nc.sync.dma_start(Wl_t[:], Wl_in[:])
            b_t = []
            for i in range(3):
                bt = constp.tile([P, 1], F32, name=f"b{i}")
                nc.sync.dma_start(bt[:], b_in[i][:])
                b_t.append(bt)
            blT_t = constp.tile([P, NC], F32)
            nc.sync.dma_start(blT_t[:], blT_in[:])

            xT_t = persist.tile([P, cfg.S_pad], F16)
            nc.sync.dma_start(xT_t[:], xT_in[:])
            idx_t = persist.tile([P, NB], I32)
            nc.sync.dma_start(idx_t[:], eidx_in[:])
            # layer-3 logits staging (batched log_softmax at the end)
            olog_t = persist.tile([CW, CH * NC], F32)

            tbl_shard = [
                dramp.tile([cfg.S_pad, P], MD, name=f"shard{i}") for i in range(3)
            ]
            tbl_full = [
                dramp.tile([cfg.T_pad, P], MD, name=f"full{i}")
                for i in range(3)
            ]

            def ag_piece(l, pc):
                nc.gpsimd.collective_compute(
                    "AllGather", mybir.AluOpType.bypass, replica_groups=rg,
                    ins=[tbl_shard[l][pc * prow:(pc + 1) * prow, :].opt()],
                    outs=[
                        tbl_full[l][
                            pc * cfg.n_cores * prow:(pc + 1) * cfg.n_cores * prow, :
                        ].opt()
                    ],
                )

            # layer 0: h1 = x @ W1, per 128-row tile, into table shard 0
            for c in range(cfg.S_pad // P):
                hp = opsump.tile([P, P], F32, name="hp", tag="o")
                nc.tensor.matmul(
                    hp[:], lhsT=xT_t[:, ts(c, P)], rhs=W_t[0][:],
                    start=True, stop=True,
                )
                hb = epip.tile([P, P], MD, name="hb")
                nc.vector.tensor_copy(hb[:], hp[:])
                nc.sync.dma_start(tbl_shard[0][ts(c, P), :], hb[:])
                if (c + 1) % ((cfg.S_pad // P) // cfg.pieces) == 0:
                    ag_piece(0, c // ((cfg.S_pad // P) // cfg.pieces))

            # message-passing layers
            for l in range(3):
                cur_psum = None
                pend = []  # (cid, aT) epilogues deferred so PE stays on msg mms

                def flush(l=None):
                    for cid, aT in pend:
                        if l < 2:
                            hp2 = opsump.tile(
                                [CW, P], F32, name="hp2", tag="o",
                                padded_shape=[P, P],
                            )
                            nc.tensor.matmul(
                                hp2[:], lhsT=aT[:], rhs=W_t[l + 1][:],
                                start=True, stop=True,
                            )
                            hb2 = epip.tile([CW, P], MD, name="hb")
                            nc.vector.tensor_copy(hb2[:], hp2[:])
                            nc.sync.dma_start(
                                tbl_shard[l + 1][cid * CW:(cid + 1) * CW, :],
                                hb2[:],
                            )
                            if (cid + 1) % cpp == 0:
                                ag_piece(l + 1, cid // cpp)
                        else:
                            lp = opsump.tile(
                                [CW, NC], F32, name="lp", tag="o",
                                padded_shape=[P, P],
                            )
                            nc.tensor.matmul(
                                lp[:], lhsT=aT[:], rhs=Wl_t[:],
                                start=True, stop=True,
                            )
                            nc.vector.tensor_tensor(
                                olog_t[:, cid * NC:(cid + 1) * NC], lp[:],
                                blT_t[:CW, :], mybir.AluOpType.add,
                            )
                    pend.clear()

                for g in range(NG):
                    gt = gatherp.tile([P, K * P], MD, name="gt")
                    nc.gpsimd.indirect_dma_start(
                        out=gt[:], out_offset=None,
                        in_=tbl_full[l][:],
                        in_offset=bass.IndirectOffsetOnAxis(
                            ap=idx_t[:, g * K:(g + 1) * K], axis=0
                        ),
                    )
                    oh = ohp.tile([P, K * CW], MD, name="oh")
                    nc.scalar.dma_start(oh[:], eoh_in[:, g * K * CW:(g + 1) * K * CW])
                    for j in range(K):
                        b = g * K + j
                        cid, first, last = blocks[b]
                        if first:
                            cur_psum = mpsump.tile([P, CW], F32, name="msg")
                        # psum[feat, w] += gathered[e, feat].T @ onehot[e, w]
                        nc.tensor.matmul(
                            cur_psum[:], lhsT=gt[:, ts(j, P)], rhs=oh[:, ts(j, CW)],
                            start=first, stop=last,
                        )
                        if not last:
                            continue
                        # bias (+ReLU) off the PE critical path, on ACT/DVE
                        aT = epip.tile([P, CW], F16, name="aT", bufs=10)
                        if l < 2:
                            nc.scalar.activation(
                                aT[:], cur_psum[:],
                                mybir.ActivationFunctionType.Relu,
                                bias=b_t[l][:, :1],
                            )
                        else:
                            nc.vector.tensor_scalar(
                                aT[:], cur_psum[:], b_t[2][:, :1], None,
                                mybir.AluOpType.add,
                            )
                        pend.append((cid, aT))
                        if len(pend) >= 8:
                            flush(l)
                flush(l)

            # batched log_softmax over all chunks: olog[p, c, k] holds logits
            v3 = olog_t[:, :].rearrange("p (c k) -> p c k", c=CH)
            mx_t = persist.tile([CW, CH], F32)
            nc.vector.reduce_max(mx_t[:], v3, axis=mybir.AxisListType.X)
            osub_t = persist.tile([CW, CH * NC], F32)
            nc.vector.tensor_tensor(
                osub_t[:, :].rearrange("p (c k) -> p c k", c=CH), v3,
                mx_t[:, :, None].broadcast_to((CW, CH, NC)),
                mybir.AluOpType.subtract,
            )
            ex_t = persist.tile([CW, CH * NC], F32)
            nc.scalar.activation(
                ex_t[:], osub_t[:], mybir.ActivationFunctionType.Exp
            )
            sums_t = persist.tile([CW, CH], F32)
            nc.vector.reduce_sum(
                sums_t[:], ex_t[:, :].rearrange("p (c k) -> p c k", c=CH),
                axis=mybir.AxisListType.X,
            )
            ln_t = persist.tile([CW, CH], F32)
            nc.scalar.activation(
                ln_t[:], sums_t[:], mybir.ActivationFunctionType.Ln
            )
            fin_t = persist.tile([CW, CH * NC], F32)
            nc.vector.tensor_tensor(
                fin_t[:, :].rearrange("p (c k) -> p c k", c=CH),
                osub_t[:, :].rearrange("p (c k) -> p c k", c=CH),
                ln_t[:, :, None].broadcast_to((CW, CH, NC)),
                mybir.AluOpType.subtract,
            )
            nc.sync.dma_start(out_t[:], fin_t[:])
    nc.compile()
    return nc


def _make_in_maps(cfg, NB, per_core, x, W1, b1, W2, b2, W3, b3, Wl, bl):
    shared = {
        "W1": np.asarray(W1, np.float16),
        "W2": np.asarray(W2, np.float16),
        "W3": np.asarray(W3, np.float16),
        "Wl": np.asarray(Wl, np.float16),
        "b1": np.asarray(b1, np.float32).reshape(P, 1),
        "b2": np.asarray(b2, np.float32).reshape(P, 1),
        "b3": np.asarray(b3, np.float32).reshape(P, 1),
        "blT": np.broadcast_to(
            np.asarray(bl, np.float32)[None, :], (P, cfg.n_class)
        ).copy(),
    }
    in_maps = []
    for c in range(cfg.n_cores):
        xs = np.zeros((P, cfg.S_pad), np.float16)
        xs[:, : cfg.S] = np.asarray(x[c * cfg.S:(c + 1) * cfg.S], np.float16).T
        in_maps.append({
            "xT": np.ascontiguousarray(xs),
            "eidx": per_core[c]["eidx"],
            "eoh": per_core[c]["eoh"],
            **shared,
        })
    return in_maps


def kernel(x, edge_index, W1, b1, W2, b2, W3, b3, Wl, bl):
    cfg = FULL
    x = np.asarray(x)
    edge_index = np.asarray(edge_index)
    NB, blocks, per_core = _preprocess(cfg, edge_index)
    nc = _build_program(cfg, NB, blocks)
    in_maps = _make_in_maps(
        cfg, NB, per_core, x, W1, b1, W2, b2, W3, b3, Wl, bl
    )
    res = run_bass_kernel_spmd(
        nc, in_maps, list(range(cfg.n_cores)),
        trace=bool(os.environ.get("GCN_TRACE")),
    )
    global LAST_RESULT
    LAST_RESULT = res
    out = np.empty((cfg.n_nodes, cfg.n_class), np.float32)
    for c in range(cfg.n_cores):
        r = np.asarray(res.results[c]["logits"])  # [CW, CH*NC]
        r = r.reshape(cfg.cw, cfg.CH, cfg.n_class).transpose(1, 0, 2)
        out[c * cfg.S:(c + 1) * cfg.S] = r.reshape(
            cfg.S_pad, cfg.n_class)[: cfg.S]
    return out



# revision 14
# speedup vs baseline: 1.1554x; 1.1554x over previous
"""3-layer GCN (PyG gcn_norm semantics) on 8 Trainium2 NeuronCores.

Sharding: nodes are range-partitioned across the 8 cores (graph parallel).
Each core owns rows [c*S, (c+1)*S) of every activation and of the
segment-sum output.  The small weight matrices are replicated.  Per layer:

  1. every core computes the dense transform h = a @ W for its own node
     range; the shards are AllGathered (in `pieces` slices, overlapped
     with compute) into a full "gather table" in each core's HBM,
  2. each core processes the edges whose *destination* lands in its range:
     an indirect DMA gathers h[src] rows (batches of GATHER_K*128 edges),
     a host-precomputed norm-weighted one-hot block
     onehot[e, w] = (dstlocal[e] == w) * norm[e] is streamed from DRAM for
     each 128-edge block (pure graph structure, fp16, reused all layers),
     and a PE matmul psum[feat, w] += gathered[e, feat].T @ onehot[e, w]
     accumulates messages into the PSUM tile of the current cw-row
     destination chunk (edges are pre-sorted by destination on the host,
     so each chunk's edges are contiguous blocks),
  3. the chunk epilogue applies bias/ReLU and the *next* layer's weight
     matmul while the data is on-chip, writing the next gather table shard.

The gather tables travel in `msg_dtype` (fp16 by default: gathers, the
one-hot stream and the message matmul run at 1-pass PE rate and half
DMA volume); all accumulation (PSUM), the epilogue, weights, and the
output stay fp32.  Chunk epilogues are batched (8 at a time) so PE is
not latency-chained through ACT between chunks.  The final layer computes logits + log_softmax per
chunk (exp/ln batched across chunks to avoid ACT LUT swaps) and each
core returns its row shard; the host concatenates.

Host-side work is limited to sharding/index preprocessing: edge
partitioning by dst range, sort by dst, degree counting, normalization
coefficients, and packing block-padded index/coefficient arrays.
All dense float math (matmuls, gathers, segment sums, activations,
log_softmax) runs on the NeuronCores.
"""

import os

import numpy as np

import concourse.bass as bass
import concourse.bacc as bacc
import concourse.mybir as mybir
import concourse.tile as tile
from concourse.bass import ts
from concourse.bass_utils import run_bass_kernel_spmd

F32 = mybir.dt.float32
F16 = mybir.dt.float16
I32 = mybir.dt.int32
P = 128  # partition dim == feature dim

LAST_RESULT = None


class Cfg:
    def __init__(self, n_cores, n_nodes, n_class, gather_k, pieces,
                 msg_dtype=mybir.dt.float8e4, cw=64):
        assert n_nodes % n_cores == 0
        self.n_cores = n_cores
        self.n_nodes = n_nodes
        self.n_class = n_class
        self.gather_k = gather_k
        self.cw = cw                         # scatter window (chunk) width
        self.S = n_nodes // n_cores          # rows per core
        self.CH = -(-self.S // cw)           # cw-row chunks per core
        self.S_pad = self.CH * cw
        assert self.S_pad % P == 0
        self.pad = self.S_pad - self.S
        self.T_pad = n_cores * self.S_pad    # padded gather-table rows
        assert self.CH % pieces == 0
        self.pieces = pieces
        self.piece_rows = self.S_pad // pieces
        self.msg_dtype = msg_dtype

    @property
    def np_msg(self):
        return np.dtype(mybir.dt.np(self.msg_dtype))


FULL = Cfg(n_cores=8, n_nodes=100000, n_class=10, gather_k=32, pieces=7)


def _table_row(cfg, i):
    """Map global node id -> row in the piece-major gather table."""
    c = i // cfg.S
    r = i - c * cfg.S
    p = r // cfg.piece_rows
    return (
        p * (cfg.n_cores * cfg.piece_rows)
        + c * cfg.piece_rows
        + (r - p * cfg.piece_rows)
    )


def _preprocess(cfg, edge_index):
    """Shard + sort edges by destination, build block-padded device arrays.

    Returns (NB, blocks, per_core) where blocks[b] = (chunk_id, first, last)
    and per_core is a list of dicts with idx/meta arrays per core.
    """
    S, CH, K = cfg.S, cfg.CH, cfg.gather_k
    n = cfg.n_nodes
    src = np.concatenate([edge_index[0], np.arange(n, dtype=np.int64)])
    dst = np.concatenate([edge_index[1], np.arange(n, dtype=np.int64)])
    deg = np.bincount(dst, minlength=n).astype(np.float64)
    dis = 1.0 / np.sqrt(deg)
    val = (dis[src] * dis[dst]).astype(np.float32)

    core = dst // S
    srcp = _table_row(cfg, src).astype(np.int32)
    # layer-1 gathers read the one-shot AllGather table (core-major rows)
    score = src // S
    srcp0 = (score * cfg.S_pad + (src - score * S)).astype(np.int32)
    dloc = dst - core * S
    chunk = dloc // cfg.cw
    w = (dloc % cfg.cw).astype(np.float32)

    G = cfg.n_cores * CH
    cc = (core * CH + chunk).astype(np.int64)
    counts = np.bincount(cc, minlength=G).reshape(cfg.n_cores, CH)
    Bc = np.maximum(1, -(-counts.max(axis=0) // P)).astype(np.int64)  # [CH]
    NB = int(Bc.sum())
    Bc[-1] += (-NB) % K
    NB = int(Bc.sum())

    chunk_off = np.zeros(CH, np.int64)
    chunk_off[1:] = np.cumsum(Bc * P)[:-1]
    L = NB * P

    # slot assignment: edges of (core, chunk) go to chunk_off[chunk] + rank;
    # within a chunk, edges are ordered by source row so the indirect
    # gather walks the table monotonically (DRAM locality)
    order = np.lexsort((srcp, cc))
    scc = cc[order]
    gstart = np.searchsorted(scc, np.arange(G))
    ranks = np.arange(len(order)) - gstart[scc]
    slots = chunk_off[scc % CH] + ranks
    cores_sorted = scc // CH

    md = cfg.np_msg
    idx_a = np.zeros((cfg.n_cores, L), np.int32)
    idx0_a = np.zeros((cfg.n_cores, L), np.int32)
    val_a = np.zeros((cfg.n_cores, L), np.float32)
    w_a = np.full((cfg.n_cores, L), -1, np.int64)
    idx_a[cores_sorted, slots] = srcp[order]
    idx0_a[cores_sorted, slots] = srcp0[order]
    val_a[cores_sorted, slots] = val[order]
    w_a[cores_sorted, slots] = w[order].astype(np.int64)

    blocks = []
    for c in range(CH):
        nb = int(Bc[c])
        for i in range(nb):
            blocks.append((c, i == 0, i == nb - 1))
    assert len(blocks) == NB

    per_core = []
    ar = np.arange(L)
    for c in range(cfg.n_cores):
        # one-hot scatter blocks: oh[e, w] = (w == wloc[e]) * val[e],
        # laid out for the device as [128 partitions(e), NB*128(b, w)]
        oh = np.zeros((L, cfg.cw), md)
        m = w_a[c] >= 0
        oh[ar[m], w_a[c][m]] = val_a[c][m].astype(md)
        oh = np.ascontiguousarray(
            oh.reshape(NB, P, cfg.cw).transpose(1, 0, 2).reshape(P, NB * cfg.cw)
        )
        # device tile layout: idx[p, b] = srcp of edge slot b*128+p
        per_core.append({
            "eidx": np.ascontiguousarray(idx_a[c].reshape(NB, P).T),
            "eidx0": np.ascontiguousarray(idx0_a[c].reshape(NB, P).T),
            "eoh": oh,
        })
    return NB, blocks, per_core


def _build_program(cfg, NB, blocks):
    nc = bacc.Bacc(
        "TRN2", target_bir_lowering=False, debug=False, num_devices=cfg.n_cores
    )
    CH, K, NC = cfg.CH, cfg.gather_k, cfg.n_class
    CW = cfg.cw
    MD = cfg.msg_dtype
    NG = NB // K
    rg = [list(range(cfg.n_cores))]
    cpp = CH // cfg.pieces          # chunks per AllGather piece
    prow = cfg.piece_rows

    # kernel I/O
    xT_in = nc.dram_tensor("xT", [P, cfg.S_pad], F16, kind="ExternalInput")
    eidx_in = nc.dram_tensor("eidx", [P, NB], I32, kind="ExternalInput")
    eidx0_in = nc.dram_tensor("eidx0", [P, NB], I32, kind="ExternalInput")
    eoh_in = nc.dram_tensor("eoh", [P, NB * CW], MD, kind="ExternalInput")
    W_in = [
        nc.dram_tensor(f"W{i + 1}", [P, P], F16, kind="ExternalInput")
        for i in range(3)
    ]
    Wl_in = nc.dram_tensor("Wl", [P, NC], F16, kind="ExternalInput")
    b_in = [
        nc.dram_tensor(f"b{i + 1}", [P, 1], F32, kind="ExternalInput")
        for i in range(3)
    ]
    blT_in = nc.dram_tensor("blT", [P, NC], F32, kind="ExternalInput")
    out_t = nc.dram_tensor("logits", [CW, CH * NC], F32, kind="ExternalOutput")

    with tile.TileContext(nc) as tc:
        with (
            tc.tile_pool(name="const", bufs=1) as constp,
            tc.tile_pool(name="persist", bufs=1) as persist,
            tc.tile_pool(name="gather", bufs=6) as gatherp,
            tc.tile_pool(name="xs", bufs=3) as xsp,
            tc.tile_pool(name="epi", bufs=3) as epip,
            tc.tile_pool(name="lsp", bufs=2) as lsp,
            tc.tile_pool(name="mpsum", bufs=3, space="PSUM") as mpsump,
            tc.tile_pool(name="opsum", bufs=4, space="PSUM") as opsump,
            tc.tile_pool(name="dram", bufs=1, space="DRAM") as dramp,
        ):
            W_t = []
            for i in range(3):
                wt = constp.tile([P, P], F16, name=f"w{i}")
                nc.sync.dma_start(wt[:], W_in[i][:])
                W_t.append(wt)
            Wl_t = constp.tile([P, NC], F16)
            nc.sync.dma_start(Wl_t[:], Wl_in[:])
            b_t = []
            for i in range(3):
                bt = constp.tile([P, 1], F32, name=f"b{i}")
                nc.sync.dma_start(bt[:], b_in[i][:])
                b_t.append(bt)
            blT_t = constp.tile([P, NC], F32)
            nc.sync.dma_start(blT_t[:], blT_in[:])

            idx_t = persist.tile([P, NB], I32)
            nc.sync.dma_start(idx_t[:], eidx_in[:])
            idx0_t = persist.tile([P, NB], I32)
            nc.sync.dma_start(idx0_t[:], eidx0_in[:])
            # one-hot scatter blocks live in SBUF for the whole kernel:
            # loaded once (overlapped with the startup barrier + layer 0)
            # instead of streamed from DRAM on every layer.
            oh_sb = persist.tile([P, NB * CW], MD)
            OHC = 8
            ohcols = NB * CW // OHC
            for j in range(OHC):
                nc.scalar.dma_start(
                    oh_sb[:, j * ohcols:(j + 1) * ohcols],
                    eoh_in[:, j * ohcols:(j + 1) * ohcols],
                )
            # layer-3 logits staging (batched log_softmax at the end)
            olog_t = persist.tile([CW, CH * NC], F32)

            tbl_shard = [
                dramp.tile([cfg.S_pad, P], MD, name=f"shard{i}") for i in range(3)
            ]
            # layer-1 table: one-shot Shared AllGather (single writer,
            # direct remote writes -> core-major row layout, uses idx0).
            # layers 2-3: piecewise Local AllGather overlapped with the
            # producing layer's chunk flushes (piece-major rows, idx).
            tbl_full = [
                dramp.tile([cfg.T_pad, P], MD, name="full0",
                           addr_space="Shared"),
                dramp.tile([cfg.T_pad, P], MD, name="full1"),
                dramp.tile([cfg.T_pad, P], MD, name="full2"),
            ]

            def ag_piece(l, pc):
                nc.gpsimd.collective_compute(
                    "AllGather", mybir.AluOpType.bypass, replica_groups=rg,
                    ins=[tbl_shard[l][pc * prow:(pc + 1) * prow, :].opt()],
                    outs=[
                        tbl_full[l][
                            pc * cfg.n_cores * prow:(pc + 1) * cfg.n_cores * prow, :
                        ].opt()
                    ],
                )

            # layer 0: h1 = x @ W1, per 128-row tile, into table shard 0
            # (xT is streamed per tile rather than kept resident, freeing
            # SBUF for the one-hot blocks)
            for c in range(cfg.S_pad // P):
                xt = xsp.tile([P, P], F16, name="xt")
                nc.sync.dma_start(xt[:], xT_in[:, ts(c, P)])
                hp = opsump.tile([P, P], F32, name="hp", tag="o")
                nc.tensor.matmul(
                    hp[:], lhsT=xt[:], rhs=W_t[0][:],
                    start=True, stop=True,
                )
                hb = epip.tile([P, P], MD, name="hb")
                nc.vector.tensor_copy(hb[:], hp[:])
                nc.sync.dma_start(tbl_shard[0][ts(c, P), :], hb[:])
            nc.gpsimd.collective_compute(
                "AllGather", mybir.AluOpType.bypass, replica_groups=rg,
                ins=[tbl_shard[0][:, :].opt()],
                outs=[tbl_full[0][:, :].opt()],
            )

            # message-passing layers
            for l in range(3):
                cur_psum = None
                pend = []  # (cid, aT) epilogues deferred so PE stays on msg mms

                def flush(l=None):
                    for cid, aT in pend:
                        if l < 2:
                            hp2 = opsump.tile(
                                [CW, P], F32, name="hp2", tag="o",
                                padded_shape=[P, P],
                            )
                            nc.tensor.matmul(
                                hp2[:], lhsT=aT[:], rhs=W_t[l + 1][:],
                                start=True, stop=True,
                            )
                            hb2 = epip.tile([CW, P], MD, name="hb")
                            nc.vector.tensor_copy(hb2[:], hp2[:])
                            nc.sync.dma_start(
                                tbl_shard[l + 1][cid * CW:(cid + 1) * CW, :],
                                hb2[:],
                            )
                            if (cid + 1) % cpp == 0:
                                ag_piece(l + 1, cid // cpp)
                        else:
                            lp = opsump.tile(
                                [CW, NC], F32, name="lp", tag="o",
                                padded_shape=[P, P],
                            )
                            nc.tensor.matmul(
                                lp[:], lhsT=aT[:], rhs=Wl_t[:],
                                start=True, stop=True,
                            )
                            nc.vector.tensor_tensor(
                                olog_t[:, cid * NC:(cid + 1) * NC], lp[:],
                                blT_t[:CW, :], mybir.AluOpType.add,
                            )
                    pend.clear()

                l_idx = idx0_t if l == 0 else idx_t
                for g in range(NG):
                    gt = gatherp.tile([P, K * P], MD, name="gt")
                    nc.gpsimd.indirect_dma_start(
                        out=gt[:], out_offset=None,
                        in_=tbl_full[l][:],
                        in_offset=bass.IndirectOffsetOnAxis(
                            ap=l_idx[:, g * K:(g + 1) * K], axis=0
                        ),
                    )
                    for j in range(K):
                        b = g * K + j
                        cid, first, last = blocks[b]
                        if first:
                            cur_psum = mpsump.tile([P, CW], F32, name="msg")
                        # psum[feat, w] += gathered[e, feat].T @ onehot[e, w]
                        nc.tensor.matmul(
                            cur_psum[:], lhsT=gt[:, ts(j, P)],
                            rhs=oh_sb[:, ts(b, CW)],
                            start=first, stop=last,
                        )
                        if not last:
                            continue
                        # bias (+ReLU) off the PE critical path, on ACT/DVE
                        aT = epip.tile([P, CW], F16, name="aT", bufs=10)
                        if l < 2:
                            nc.scalar.activation(
                                aT[:], cur_psum[:],
                                mybir.ActivationFunctionType.Relu,
                                bias=b_t[l][:, :1],
                            )
                        else:
                            nc.vector.tensor_scalar(
                                aT[:], cur_psum[:], b_t[2][:, :1], None,
                                mybir.AluOpType.add,
                            )
                        pend.append((cid, aT))
                        if len(pend) >= 8:
                            flush(l)
                flush(l)

            # batched log_softmax over all chunks: olog[p, c, k] holds logits
            # (subtractions run in place on olog to save SBUF)
            v3 = olog_t[:, :].rearrange("p (c k) -> p c k", c=CH)
            mx_t = persist.tile([CW, CH], F32)
            nc.vector.reduce_max(mx_t[:], v3, axis=mybir.AxisListType.X)
            nc.vector.tensor_tensor(
                v3, v3,
                mx_t[:, :, None].broadcast_to((CW, CH, NC)),
                mybir.AluOpType.subtract,
            )
            ex_t = persist.tile([CW, CH * NC], F32)
            nc.scalar.activation(
                ex_t[:], olog_t[:], mybir.ActivationFunctionType.Exp
            )
            sums_t = persist.tile([CW, CH], F32)
            nc.vector.reduce_sum(
                sums_t[:], ex_t[:, :].rearrange("p (c k) -> p c k", c=CH),
                axis=mybir.AxisListType.X,
            )
            ln_t = persist.tile([CW, CH], F32)
            nc.scalar.activation(
                ln_t[:], sums_t[:], mybir.ActivationFunctionType.Ln
            )
            nc.vector.tensor_tensor(
                v3, v3,
                ln_t[:, :, None].broadcast_to((CW, CH, NC)),
                mybir.AluOpType.subtract,
            )
            nc.sync.dma_start(out_t[:], olog_t[:])
    nc.compile()
    return nc


def _make_in_maps(cfg, NB, per_core, x, W1, b1, W2, b2, W3, b3, Wl, bl):
    shared = {
        "W1": np.asarray(W1, np.float16),
        "W2": np.asarray(W2, np.float16),
        "W3": np.asarray(W3, np.float16),
        "Wl": np.asarray(Wl, np.float16),
        "b1": np.asarray(b1, np.float32).reshape(P, 1),
        "b2": np.asarray(b2, np.float32).reshape(P, 1),
        "b3": np.asarray(b3, np.float32).reshape(P, 1),
        "blT": np.broadcast_to(
            np.asarray(bl, np.float32)[None, :], (P, cfg.n_class)
        ).copy(),
    }
    in_maps = []
    for c in range(cfg.n_cores):
        xs = np.zeros((P, cfg.S_pad), np.float16)
        xs[:, : cfg.S] = np.asarray(x[c * cfg.S:(c + 1) * cfg.S], np.float16).T
        in_maps.append({
            "xT": np.ascontiguousarray(xs),
            "eidx": per_core[c]["eidx"],
            "eidx0": per_core[c]["eidx0"],
            "eoh": per_core[c]["eoh"],
            **shared,
        })
    return in_maps


def kernel(x, edge_index, W1, b1, W2, b2, W3, b3, Wl, bl):
    cfg = FULL
    x = np.asarray(x)
    edge_index = np.asarray(edge_index)
    NB, blocks, per_core = _preprocess(cfg, edge_index)
    nc = _build_program(cfg, NB, blocks)
    in_maps = _make_in_maps(
        cfg, NB, per_core, x, W1, b1, W2, b2, W3, b3, Wl, bl
    )
    res = run_bass_kernel_spmd(
        nc, in_maps, list(range(cfg.n_cores)),
        trace=bool(os.environ.get("GCN_TRACE")),
    )
    global LAST_RESULT
    LAST_RESULT = res
    out = np.empty((cfg.n_nodes, cfg.n_class), np.float32)
    for c in range(cfg.n_cores):
        r = np.asarray(res.results[c]["logits"])  # [CW, CH*NC]
        r = r.reshape(cfg.cw, cfg.CH, cfg.n_class).transpose(1, 0, 2)
        out[c * cfg.S:(c + 1) * cfg.S] = r.reshape(
            cfg.S_pad, cfg.n_class)[: cfg.S]
    return out

